# revision 21
# baseline (speedup 1.0000x reference)
"""CRF energy kernel for Trainium2, SPMD across 8 NeuronCores.

Computes energy = x @ kernel + bias + start_mask*left_boundary + end_mask*right_boundary
  x: [64, 512, 1024] f32, kernel: [1024, 128], out: [64, 512, 128] f32.

Strategy: data-parallel over batch (8 batches/core -> x shard [4096, 1024]).
Per core:
  - DMA x tiles [128 t, 1024 d] HBM->SBUF with f32->bf16 cast (SWDGE inline cast).
  - PE-transpose each [128 t, 128 d] block -> xT [128 d, 128 t] (bf16, via PSUM+DVE copy).
  - 8 accumulating bf16 matmuls per output tile: psum[t,u] += xT_k.T @ W_k.
  - Boundary/bias terms folded in as one extra matmul: psum += S_tile.T @ V where
    S^T rows = [start_mask, end_mask, ones, 0...] (padded to 128), V rows =
    [left_boundary, right_boundary, bias, 0...].
  - DVE evict psum -> SBUF f32, batched DMA to DRAM.
"""

import numpy as np
import ml_dtypes

import concourse.bass as bass
import concourse.mybir as mybir
import concourse.tile as tile
from concourse import bacc
from concourse.bass_utils import run_bass_kernel_spmd
from concourse.masks import make_identity
from contextlib import ExitStack

B, T, D, U = 64, 512, 1024, 128
NCORES = 8
MB = B // NCORES            # batches per core
M = MB * T                  # 4096 rows per core
P = 128
KT = D // P                 # 8 k-tiles
NT = M // P                 # 32 t-tiles per core
XB = 2                      # t-tiles per x-load DMA
OB = 4                      # t-tiles per output DMA

BF16 = mybir.dt.bfloat16
F32 = mybir.dt.float32

_CACHE = {}
LAST_RESULTS = None


def build_nc():
    nc = bacc.Bacc(target_bir_lowering=False)
    x = nc.declare_dram_parameter("x", [M, D], F32, isOutput=False)
    w = nc.declare_dram_parameter("w", [D, U], BF16, isOutput=False)
    sv = nc.declare_dram_parameter("sv", [4, U], BF16, isOutput=False)
    st = nc.declare_dram_parameter("st", [4, M], BF16, isOutput=False)
    idm = nc.declare_dram_parameter("idm", [P, P], BF16, isOutput=False)
    out = nc.declare_dram_parameter("out", [M, U], F32, isOutput=True)

    with ExitStack() as ctx:
        tc = ctx.enter_context(tile.TileContext(nc))
        consts = ctx.enter_context(tc.tile_pool(name="consts", bufs=1))
        xpool = ctx.enter_context(tc.tile_pool(name="xpool", bufs=18))
        tpool = ctx.enter_context(tc.tile_pool(name="tpool", bufs=3))
        opool = ctx.enter_context(tc.tile_pool(name="opool", bufs=2))
        pps = ctx.enter_context(tc.tile_pool(name="pps", bufs=5, space="PSUM"))
        pout = ctx.enter_context(tc.tile_pool(name="pout", bufs=3, space="PSUM"))

        # constants
        w_sb = consts.tile([P, KT, U], BF16)           # [ki, ko, u]
        nc.sync.dma_start(out=w_sb, in_=w[:, :].rearrange("(ko ki) u -> ki ko u", ki=P))
        sv_sb = consts.tile([P, U], BF16)              # [r(pad 128), u]
        nc.vector.memset(sv_sb, 0.0)
        nc.sync.dma_start(out=sv_sb[0:4, :], in_=sv[:, :])
        st_sb = consts.tile([P, NT, P], BF16)          # [r(pad 128), tile, t]
        nc.vector.memset(st_sb, 0.0)
        nc.sync.dma_start(out=st_sb[0:4], in_=st[:, :].rearrange("r (i t) -> r i t", t=P))
        ident = consts.tile([P, P], BF16)
        nc.sync.dma_start(out=ident, in_=idm[:, :])

        TG = 4                                         # transposes per psum bank
        # x-load chunk sizes (t-tiles per DMA): small head chunks so compute
        # starts early, small tail chunks so the drain is short.
        sizes = [1, 1, 2] + [2] * 13 + [1, 1]
        assert sum(sizes) == NT
        # output DMA groups: 4-tile groups, small tail groups for a short drain
        og = [4] * 7 + [2, 1, 1]
        assert sum(og) == NT
        ogroup_start, ogroup_end = {}, {}
        t0 = 0
        for n in og:
            for t in range(t0, t0 + n):
                ogroup_start[t] = t0
                ogroup_end[t] = t0 + n - 1
            t0 += n
        state = {"po": None}

        def tp_block(i, xa, j):
            # 8 PE transposes + 2 DVE copies -> xT tile
            xt = tpool.tile([P, KT, P], BF16, tag="xt", name="xt")  # [d, k, t]
            for kg in range(KT // TG):
                pt = pps.tile([P, TG * P], BF16, tag="pt", name="pt")
                for kk in range(TG):
                    k = kg * TG + kk
                    nc.tensor.transpose(pt[:, kk * P:(kk + 1) * P],
                                        xa[:, j, k * P:(k + 1) * P], ident)
                if kg % 2 == 0:
                    nc.vector.tensor_copy(out=xt[:, kg * TG:(kg + 1) * TG, :], in_=pt)
                else:
                    nc.scalar.copy(out=xt[:, kg * TG:(kg + 1) * TG, :], in_=pt)
            return xt

        def mm_block(i, xt):
            # 9 accumulating matmuls into a quarter of the shared psum bank,
            # then (on group end) evict + store.
            if i == ogroup_start[i]:
                state["po"] = pout.tile([P, OB * U], F32, tag="po", name="po")
            po = state["po"]
            oj = i - ogroup_start[i]
            ps = po[:, oj * U:(oj + 1) * U]
            for k in range(KT):
                nc.tensor.matmul(ps, lhsT=xt[:, k, :], rhs=w_sb[:, k, :],
                                 start=(k == 0), stop=False)
            nc.tensor.matmul(ps, lhsT=st_sb[:, i, :], rhs=sv_sb,
                             start=False, stop=True)
            if i == ogroup_end[i]:
                i0 = ogroup_start[i]
                n = i - i0 + 1
                ob = opool.tile([P, OB, U], F32, tag="ob", name="ob")
                nc.vector.tensor_copy(out=ob[:, :n, :], in_=po[:, :n * U])
                dst = out[i0 * P:(i + 1) * P, :]
                nc.sync.dma_start(out=dst.rearrange("(a t) u -> t a u", t=P),
                                  in_=ob[:, :n, :])

        # Software-pipeline PE by one tile: transposes for tile i+1 are
        # emitted before the matmuls of tile i, so PE never stalls on the
        # DVE psum->sbuf copy of its own tile.
        pending = None                                 # (i, xt)
        i = 0
        for sz in sizes:
            xa = xpool.tile([P, sz, D], BF16, tag="xa")  # [t, a, d]
            src = x[i * P:(i + sz) * P, :]
            nc.gpsimd.dma_start(out=xa, in_=src.rearrange("(a t) d -> t a d", t=P))
            for j in range(sz):
                xt = tp_block(i, xa, j)
                if pending is not None:
                    mm_block(*pending)
                pending = (i, xt)
                i += 1
        mm_block(*pending)
    nc.finalize()
    return nc


def _shift_right(m):
    z = np.zeros_like(m[:, :1])
    return np.concatenate([z, m[:, :-1]], axis=1)


def _shift_left(m):
    z = np.zeros_like(m[:, :1])
    return np.concatenate([m[:, 1:], z], axis=1)


def kernel(x, mask, kernel, bias, left_boundary, right_boundary):
    global LAST_RESULTS
    x = np.asarray(x, dtype=np.float32)
    mask = np.asarray(mask)
    kern = np.asarray(kernel, dtype=np.float32)
    bias = np.asarray(bias, dtype=np.float32)
    lb = np.asarray(left_boundary, dtype=np.float32)
    rb = np.asarray(right_boundary, dtype=np.float32)

    if "nc" not in _CACHE:
        _CACHE["nc"] = build_nc()
    nc = _CACHE["nc"]

    bf = ml_dtypes.bfloat16
    w_b = kern.astype(bf)                                    # [D, U]
    sv_b = np.stack([lb, rb, bias, np.zeros_like(bias)]).astype(bf)  # [4, U]

    m = mask.astype(np.float32)                              # [B, T]
    sm = (m > _shift_right(m)).astype(np.float32)
    em = (_shift_left(m) > m).astype(np.float32)

    in_maps = []
    for c in range(NCORES):
        xs = x[c * MB:(c + 1) * MB].reshape(M, D)
        sm_c = sm[c * MB:(c + 1) * MB].reshape(M)
        em_c = em[c * MB:(c + 1) * MB].reshape(M)
        st_c = np.stack([sm_c, em_c, np.ones(M, np.float32),
                         np.zeros(M, np.float32)]).astype(bf)  # [4, M]
        in_maps.append({"x": xs, "w": w_b, "sv": sv_b, "st": st_c,
                        "idm": np.eye(P, dtype=np.float32).astype(bf)})

    res = run_bass_kernel_spmd(nc, in_maps, core_ids=list(range(NCORES)))
    LAST_RESULTS = res
    outs = [np.asarray(res.results[c]["out"], dtype=np.float32) for c in range(NCORES)]
    return np.concatenate(outs, axis=0).reshape(B, T, U)
